# revision 1
# baseline (speedup 1.0000x reference)
"""RoIAlignRotated Trainium2 kernel.

Strategy: rois are sharded across 8 NeuronCores (125 rois each); every core
holds a full replica of a precomputed "neighborhood table" in HBM:
nb[b,y,x] = the 2x2 bilinear neighborhood [f(y,x), f(y,x+1), f(y+1,x),
f(y+1,x+1)] of channels-last features, fp16 (131072 rows x 2KB). One
indirect-DMA read per sampling point fetches all four bilinear taps.

Device layout per 128-bin tile: partition p = b0*4 + s holds bin
(j*32+b0)'s sampling point s in free-slot j (j in 0..3). The bilinear
weights (x valid mask x 1/4 grid mean) arrive as dense [128,16] vectors and
the DVE expands them against a constant 0/1 bin-selection mask into
per-(tile,j,nb) [128,32] fp16 stationary matrices, so the PE does the
entire weighted reduction: out[b0,j,:] = sum_{s,nb} w * G[p, nb*256:+256],
accumulating the 4 neighborhood taps in PSUM. ACT evacuates PSUM per
sampling column and stores 128 bins per DMA.

Measured floor (~290us/8 cores): the stream of 192 indirect-DMA gather ops
per core (1.09us Q7 descriptor-gen + 0.31us dispatch gap each, co-limited
by ~1.03us/op SDMA random-2KB drain). Verified no-ops: extra SWDGE queues,
single_packet, deeper tile buffers, larger descriptor ring, fewer SDMA
bytes.
"""

import os

import numpy as np

# Problem constants (hardcoded per contract; kernel.py must be self-contained).
B, C, H, W = 2, 256, 256, 256
N_ROIS = 1000
OH = OW = 7
GH = GW = 2
NSAMP = GH * GW                       # 4 sampling points per bin
SPATIAL_SCALE = 0.25
NCORES = 8

NROI_PC = N_ROIS // NCORES            # 125 rois per core
BINS_PC = NROI_PC * OH * OW           # 6125 output bins per core
NTILES = (BINS_PC + 127) // 128       # 48 tiles of 128 bins
BINS_PAD = NTILES * 128               # 6144
ROWS = B * H * W                      # 131072 neighborhood-table rows

_CACHE = {}
LAST_RESULTS = None  # BassKernelResults of the most recent run (for profiling)


def _build_bass():
    import concourse.bacc as bacc
    import concourse.bass as bass
    import concourse.mybir as mybir
    import concourse.tile as tile

    f32 = mybir.dt.float32
    f16 = mybir.dt.float16
    i32 = mybir.dt.int32

    nq = int(os.environ.get("ROI_NSWQ", "1"))
    nc = bacc.Bacc(
        "TRN2",
        target_bir_lowering=False,
        name="roialignrot",
        num_swdge_queues=nq,
        dynamic_dma_scratch_size=int(os.environ.get("ROI_DMA_SCRATCH", "16384")),
    )
    feat_d = nc.dram_tensor("feat", [ROWS, 4 * C], f16, kind="ExternalInput")
    idx_d = nc.dram_tensor("idx", [128, NTILES, NSAMP], i32, kind="ExternalInput")
    # dense per-(p, tile, j, nb) weights; expanded on-device to stationaries
    wts_d = nc.dram_tensor("wts", [128, NTILES, 16], f32, kind="ExternalInput")
    out_d = nc.dram_tensor("out", [BINS_PAD, C], f32, kind="ExternalOutput")

    # 0/1 mask: column p//4 of row p (bin-selection structure of the
    # stationaries; the weight value is multiplied in on the DVE)
    sel_np = (np.arange(128)[:, None] // NSAMP == np.arange(32)[None, :]).astype(
        np.float16
    )
    sel_d = nc.inline_tensor(sel_np, name="sel01")

    with tile.TileContext(nc) as tc:
        with (
            tc.tile_pool(name="const", bufs=1) as constp,
            tc.tile_pool(name="io", bufs=6) as iop,
            tc.tile_pool(name="big", bufs=3) as bigp,
            tc.tile_pool(name="stage", bufs=4) as stagep,
            tc.tile_pool(name="psum", bufs=4, space="PSUM") as psump,
        ):
            # all offsets + weights in two DMAs (96KB + 192KB) up front, so
            # the gather stream never waits on per-tile loads
            sel01_t = constp.tile([128, 32], f16)
            nc.sync.dma_start(sel01_t[:], sel_d[:])
            idx_all = constp.tile([128, NTILES, NSAMP], i32)
            nc.sync.dma_start(idx_all[:], idx_d[:])
            wd_all = constp.tile([128, NTILES, 16], f32)
            nc.sync.dma_start(wd_all[:], wts_d[:])

            for t in range(NTILES):
                Gj = [
                    bigp.tile([128, 4 * C], f16, tag=f"g{j}", name=f"g{j}_{t}")
                    for j in range(NSAMP)
                ]
                for j in range(NSAMP):
                    # Gj[p, :] = nbhd row idx[p, t, j] (2KB: 4 taps x 256 ch)
                    gi = nc.gpsimd.indirect_dma_start(
                        out=Gj[j][:],
                        out_offset=None,
                        in_=feat_d[:],
                        in_offset=bass.IndirectOffsetOnAxis(
                            ap=idx_all[:, t, j:j + 1], axis=0
                        ),
                    )
                    q = (t * NSAMP + j) % nq
                    if q:
                        gi.queue = f"qPoolDynamic{q}"
                # expand dense weights into [128, 32] stationaries (DVE idle)
                selw_x = iop.tile([128, 16, 32], f16)
                for k in range(16):
                    nc.vector.tensor_scalar_mul(
                        selw_x[:, k, :], sel01_t[:], wd_all[:, t, k:k + 1]
                    )
                ps = psump.tile([32, NSAMP, C], f32)
                stage = stagep.tile([32, NSAMP, C], f32)
                for j in range(NSAMP):
                    for nb in range(4):
                        nc.tensor.matmul(
                            out=ps[:, j, :],
                            lhsT=selw_x[:, 4 * j + nb, :],
                            rhs=Gj[j][:, nb * C:(nb + 1) * C],
                            start=(nb == 0),
                            stop=(nb == 3),
                        )
                    # per-j PSUM -> SBUF so the tail chain is short
                    nc.scalar.activation(
                        stage[:, j, :], ps[:, j, :],
                        func=mybir.ActivationFunctionType.Copy,
                    )
                # DRAM row = t*128 + j*32 + b0 ; SBUF stage is [b0, j, c]
                dview = out_d[t * 128:(t + 1) * 128, :].rearrange(
                    "(j b) c -> b j c", j=NSAMP, b=32
                )
                nc.scalar.dma_start(dview, stage[:])

    nc.compile()
    return nc


def _get_nc():
    if "nc" not in _CACHE:
        _CACHE["nc"] = _build_bass()
    return _CACHE["nc"]


def _build_nbhd_table(features):
    """fp16 channels-last 2x2-neighborhood table [B*H*W, 4*C]."""
    f = features.transpose(0, 2, 3, 1).astype(np.float16)  # [B, H, W, C]
    nb = np.empty((B, H, W, 4, C), np.float16)
    xp = np.minimum(np.arange(W) + 1, W - 1)
    yp = np.minimum(np.arange(H) + 1, H - 1)
    nb[:, :, :, 0, :] = f
    nb[:, :, :, 1, :] = f[:, :, xp, :]
    nb[:, :, :, 2, :] = f[:, yp, :, :]
    nb[:, :, :, 3, :] = f[:, yp][:, :, xp]
    return nb.reshape(ROWS, 4 * C)


def _indices_weights(rois):
    """Per-bin sampling-point rows and folded weights, mirroring the
    reference math in float32.

    Returns idx [NBINS, 4] int32 and wts [NBINS, 4, 4] f32 (per-tap)."""
    f = np.float32
    b = rois[:, 0].astype(np.int32)
    cx = rois[:, 1] * f(SPATIAL_SCALE)
    cy = rois[:, 2] * f(SPATIAL_SCALE)
    rw = np.maximum(rois[:, 3] * f(SPATIAL_SCALE), f(0.0))
    rh = np.maximum(rois[:, 4] * f(SPATIAL_SCALE), f(0.0))
    theta = rois[:, 5]

    bin_h = rh / f(OH)
    bin_w = rw / f(OW)
    ph = np.arange(OH, dtype=f)
    pw = np.arange(OW, dtype=f)
    iy = (np.arange(GH, dtype=f) + f(0.5)) / f(GH)
    ix = (np.arange(GW, dtype=f) + f(0.5)) / f(GW)

    yy = (-rh / f(2.0))[:, None, None] + bin_h[:, None, None] * (
        ph[None, :, None] + iy[None, None, :]
    )  # [N, OH, GH]
    xx = (-rw / f(2.0))[:, None, None] + bin_w[:, None, None] * (
        pw[None, :, None] + ix[None, None, :]
    )  # [N, OW, GW]

    yyf = yy[:, :, None, :, None]  # [N, OH, 1, GH, 1]
    xxf = xx[:, None, :, None, :]  # [N, 1, OW, 1, GW]
    cosv = np.cos(theta)[:, None, None, None, None]
    sinv = np.sin(theta)[:, None, None, None, None]
    y = yyf * cosv - xxf * sinv + cy[:, None, None, None, None]  # [N,OH,OW,GH,GW]
    x = yyf * sinv + xxf * cosv + cx[:, None, None, None, None]

    valid = (y > f(-1.0)) & (y < f(H)) & (x > f(-1.0)) & (x < f(W))
    yc = np.clip(y, f(0.0), f(H - 1))
    xc = np.clip(x, f(0.0), f(W - 1))
    y0 = np.minimum(np.floor(yc).astype(np.int32), H - 1)
    x0 = np.minimum(np.floor(xc).astype(np.int32), W - 1)
    ly = yc - y0.astype(f)
    lx = xc - x0.astype(f)
    hy = f(1.0) - ly
    hx = f(1.0) - lx
    vm = valid.astype(f) * f(0.25)  # fold the mean over the GH*GW grid samples

    # tap weights; the table's clamped duplicate taps absorb the x1==x0 /
    # y1==y0 edge cases exactly
    w = np.stack([hy * hx, hy * lx, ly * hx, ly * lx], axis=-1) * vm[..., None]
    idx = b[:, None, None, None, None] * (H * W) + y0 * W + x0

    nbins = N_ROIS * OH * OW
    idx = idx.reshape(nbins, NSAMP).astype(np.int32)
    wts = w.reshape(nbins, NSAMP, 4).astype(f)
    return idx, wts


def _make_in_maps(features, rois):
    feat = _build_nbhd_table(features)
    idx_all, wts_all = _indices_weights(rois)
    in_maps = []
    for core in range(NCORES):
        lo = core * BINS_PC
        hi = lo + BINS_PC
        idx_c = np.zeros((BINS_PAD, NSAMP), np.int32)
        wts_c = np.zeros((BINS_PAD, NSAMP, 4), np.float32)
        idx_c[:BINS_PC] = idx_all[lo:hi]
        wts_c[:BINS_PC] = wts_all[lo:hi]
        # bin = t*128 + j*32 + b0 ; partition p = b0*4 + s
        idx_t = idx_c.reshape(NTILES, NSAMP, 32, NSAMP)     # [t, j, b0, s]
        idx_t = idx_t.transpose(2, 3, 0, 1)                 # [b0, s, t, j]
        # wts[p=(b0,s), t, (j,nb)] = w[bin(t,j,b0), s, nb]
        wts_t = wts_c.reshape(NTILES, NSAMP, 32, NSAMP, 4)  # [t, j, b0, s, nb]
        wts_t = wts_t.transpose(2, 3, 0, 1, 4)              # [b0, s, t, j, nb]
        in_maps.append(
            {
                "feat": feat,
                "idx": np.ascontiguousarray(idx_t.reshape(128, NTILES, NSAMP)),
                "wts": np.ascontiguousarray(
                    wts_t.astype(np.float16).astype(np.float32).reshape(128, NTILES, 16)
                ),
            }
        )
    return in_maps


def _unpack_out(res_out):
    return res_out[:BINS_PC].reshape(NROI_PC, OH, OW, C).transpose(0, 3, 1, 2)


def _ensure_ntff_hook():
    """bass_utils' trace=True path imports antenv.axon_hooks, which this
    image lacks — shim it (and install the libaxon NTFF hook) best-effort."""
    import sys
    import types

    if "antenv.axon_hooks" in sys.modules:
        return
    try:
        import antenv

        mod = types.ModuleType("antenv.axon_hooks")
        _hook = [None]
        mod.set_axon_ntff_profile_hook = lambda h: _hook.__setitem__(0, h)
        mod.get_axon_ntff_profile_hook = lambda: _hook[0]
        sys.modules["antenv.axon_hooks"] = mod
        antenv.axon_hooks = mod
        from trn_agent_boot.trn_boot import _ntff_profile_via_ctypes

        mod.set_axon_ntff_profile_hook(
            _ntff_profile_via_ctypes("/opt/axon/libaxon_pjrt.so")
        )
    except Exception:
        pass


def kernel(features, rois, out_w=7, out_h=7):
    global LAST_RESULTS
    from concourse.bass_utils import run_bass_kernel_spmd

    _ensure_ntff_hook()

    features = np.asarray(features, dtype=np.float32)
    rois = np.asarray(rois, dtype=np.float32)
    assert int(out_w) == OW and int(out_h) == OH
    assert features.shape == (B, C, H, W) and rois.shape == (N_ROIS, 6)

    in_maps = _make_in_maps(features, rois)
    nc = _get_nc()
    res = run_bass_kernel_spmd(
        nc,
        in_maps,
        core_ids=list(range(NCORES)),
        trace=bool(int(os.environ.get("ROI_TRACE", "0"))),
    )
    LAST_RESULTS = res
    outs = [_unpack_out(r["out"]) for r in res.results]
    return np.ascontiguousarray(np.concatenate(outs, axis=0))

